# revision 28
# baseline (speedup 1.0000x reference)
"""BioMech feature extraction on Trainium2: 8 NeuronCores, pure data-parallel.

Self-contained: takes full inputs foot/shank/thigh [8192, 12, 256] fp32,
returns [8192, 44] fp32 feature matrix matching the reference stack order.

Strategy per core (1024 samples, 8 blocks of 128 partitions), fp16 inputs
(host-downcast; well within the 2e-2 gate):
  - Host packs the 22 used channels -> X [B, 22, 256] fp16 and additionally
    ships the two foot-z channels pre-transposed (t on partitions) so the
    DFT runs straight off DMA with data-as-stationary matmuls (no on-chip
    transposes, no PSUM->SBUF copies of data).
  - FFT features via merged DFT weights [C|S|C*sqrt(k)|S*sqrt(k)] (512 cols)
    at fp16 full PE rate; sum x^2 from Parseval.
  - Peaks via i16 sign-mask abs (DVE 4x) + fp16 tensor_tensor max tree (2x)
    + small grouped tail reduces: ~2x faster than a flat 1x abs-max reduce.
  - Shank-z variance via one interleaved-pair bn_stats call per block (the
    even/odd stat split yields exact per-channel mean/M2 for two channels).
  - Gyro variances bypass the DVE entirely: the host also ships the 18 gyro
    channels t-transposed (fp16); per-channel Sx comes from FD=1 ones-matmuls
    on the PE (data as stationary, so the output lands sample-major), and
    Sxx from the same matmuls over on-chip squares (split DVE/ACT by chunk).
  - Raw moments Sx3/Sx4 via fp16 powers of the transposed fz + FD=1
    ones-matmuls into PSUM columns; central-moment combine in final phase.
  - Half-wave |x| sums (decay) likewise via ones-matmuls on |fz_T|.
  - impact/zcr/vib: GPS products + ACT sign/abs accumulators.
"""

import contextlib

import numpy as np

import concourse.bacc as bacc
import concourse.bass as _bass
import concourse.tile as tile
import concourse.mybir as mybir
from concourse.bass_utils import run_bass_kernel_spmd

F32 = mybir.dt.float32
F16 = mybir.dt.float16
FP8 = mybir.dt.float8e4
I16 = mybir.dt.int16
AF = mybir.ActivationFunctionType
ALU = mybir.AluOpType
AX = mybir.AxisListType

N_CORES = 8
B_FULL = 8192
T = 256
P = 128
BC = B_FULL // N_CORES          # 1024 samples per core
NBLK = BC // P                  # 8 blocks
NCH = 22
NBINS = 129
HF_BIN = 60
NW = 512                        # merged DFT weight columns
EPS = 1e-6
NS = NBLK * 2                   # (block, side) stat slots

# packed channel order:
# 0 fzL, 1 fzR, 2 szL, 3 szR, 4:7 fgL, 7:10 fgR, 10:13 sgL, 13:16 sgR,
# 16:19 tgL, 19:22 tgR
SRC = [("foot", 2), ("foot", 8), ("shank", 2), ("shank", 8),
       ("foot", 3), ("foot", 4), ("foot", 5), ("foot", 9), ("foot", 10), ("foot", 11),
       ("shank", 3), ("shank", 4), ("shank", 5), ("shank", 9), ("shank", 10), ("shank", 11),
       ("thigh", 3), ("thigh", 4), ("thigh", 5), ("thigh", 9), ("thigh", 10), ("thigh", 11)]

# bn pairing: single interleaved call for the shank-z pair; gyro stats come
# from the fp16 t-layout PE path instead.
BN_A = [2]
BN_B = [3]
NG = 18                         # gyro channels (packed 4..21)
SQ_V = 12                       # gyro channels squared on DVE (rest on ACT)


def build_consts():
    t = np.arange(T, dtype=np.float64)
    k = np.arange(NBINS, dtype=np.float64)
    ang = 2.0 * np.pi * np.outer(t, k) / T
    C = np.cos(ang)                      # [256, 129] k=0..128
    S = np.sin(ang)
    sk = np.sqrt(k)
    # merged: [C(0..128) | S(1..128) | C*sqrt(k)(1..128) | S*sqrt(k)(1..127)]
    w = np.concatenate([C, S[:, 1:129], (C * sk)[:, 1:129],
                        (S * sk)[:, 1:128]], axis=1)       # [256, 512]
    assert w.shape[1] == NW
    w = np.ascontiguousarray(w.reshape(2, P, NW), dtype=np.float16)
    ones = np.ones((P, 1), dtype=np.float16)
    import ml_dtypes
    ones8 = np.ones((P, 1), dtype=ml_dtypes.float8_e4m3fn)
    return {"w": w, "ones": ones, "ones8": ones8}


def build_nc():
    nc = bacc.Bacc("TRN2", target_bir_lowering=False, debug=False,
                   num_devices=N_CORES)
    x_d = nc.dram_tensor("x", [BC, NCH, T], F16, kind="ExternalInput")
    xt_d = nc.dram_tensor("xt", [NBLK, P, 2, 2, P], F16, kind="ExternalInput")
    g8_d = nc.dram_tensor("g8", [NBLK, P, 2, NG, P], F16,
                          kind="ExternalInput")
    w_d = nc.dram_tensor("w", [2, P, NW], F16, kind="ExternalInput")
    on_d = nc.dram_tensor("ones", [P, 1], F16, kind="ExternalInput")
    on8_d = nc.dram_tensor("ones8", [P, 1], FP8, kind="ExternalInput")
    out_d = nc.dram_tensor("out", [BC, 44], F32, kind="ExternalOutput")

    with tile.TileContext(nc) as tc:
        _body(tc, x_d, xt_d, g8_d, w_d, on_d, on8_d, out_d)
    nc.compile()
    return nc


def _bn_interleaved(nc, V, out6, xpair):
    """bn_stats over two channels interleaved: even stats = first channel,
    odd stats = second. xpair: AP [p, 2, T] (c-major); out6: AP [p, 6]."""
    xi = _bass.AP(tensor=xpair.tensor, offset=xpair.offset,
                  ap=[xpair.ap[0], [1, T], [T, 2]])
    V.add_instruction(mybir.InstBNStats(
        name=f"I-{nc.next_id()}",
        ins=[V.lower_ap(xi)],
        outs=[V.lower_ap(out6)]))


def _body(tc, x_d, xt_d, g8_d, w_d, on_d, on8_d, out_d):
    nc = tc.nc
    ctx = contextlib.ExitStack()
    with ctx:
        pers = ctx.enter_context(tc.tile_pool(name="pers", bufs=1))
        p_in = ctx.enter_context(tc.tile_pool(name="xin", bufs=4))
        p_xt = ctx.enter_context(tc.tile_pool(name="xtin", bufs=3))
        p_g8 = ctx.enter_context(tc.tile_pool(name="g8in", bufs=3))
        p_gsq = ctx.enter_context(tc.tile_pool(name="gsq", bufs=2))
        p_abs = ctx.enter_context(tc.tile_pool(name="abs", bufs=2))
        p_tree = ctx.enter_context(tc.tile_pool(name="tree", bufs=2))
        p_pow = ctx.enter_context(tc.tile_pool(name="pow", bufs=2))
        p_scr = ctx.enter_context(tc.tile_pool(name="scr", bufs=2))
        p_junk = ctx.enter_context(tc.tile_pool(name="junk", bufs=4))
        p_small = ctx.enter_context(tc.tile_pool(name="small", bufs=3))
        p_psf = ctx.enter_context(tc.tile_pool(name="psf", bufs=3, space="PSUM"))
        p_psk = ctx.enter_context(tc.tile_pool(name="psk", bufs=2, space="PSUM"))
        fin = ctx.enter_context(tc.tile_pool(name="fin", bufs=1))

        V = nc.vector
        SC = nc.scalar
        G = nc.gpsimd
        TE = nc.tensor

        # ---- constants ----
        w_sb = pers.tile([P, 2, NW], F16, tag="w", name="w_sb")
        on_sb = pers.tile([P, 1], F16, tag="ones", name="on_sb")
        on8_sb = pers.tile([P, 1], FP8, tag="ones8", name="on8_sb")
        nc.sync.dma_start(out=w_sb[:], in_=w_d.ap().rearrange("j p n -> p j n"))
        nc.sync.dma_start(out=on_sb[:], in_=on_d.ap())
        nc.sync.dma_start(out=on8_sb[:], in_=on8_d.ap())

        def stat(tag, n=NS):
            return pers.tile([P, n], F32, tag=tag, name=tag)

        tot_s = stat("tot")     # sum power (129 cos bins + 128 sin bins)
        scn_s = stat("scn")     # sum k*power
        hf_s = stat("hf")       # sum power k>=60
        muN_s = stat("muN", 2 * NS)  # [mu, aN] per (b, side), POSITIVE mu
        imp_s = stat("imp")     # sum sign(|fz| - 0.3pk)
        zc_s = stat("zc")       # sum sign(x_t * x_{t+1})
        vib_s = stat("vib")     # sum |diff sz|
        # psKG cols: 0,1 habs0 (L,R); 2,3 habs1 (L,R); 4,5 Sx3; 6,7 Sx4;
        # 8:26 gyro Sx per channel; 26:44 gyro Sxx per channel
        mg_s = pers.tile([P, NBLK, 44], F32, tag="mg", name="mg_s")
        bnS_s = pers.tile([P, NBLK, 6], F32, tag="bnS", name="bnS")
        out_t = pers.tile([P, NBLK, 44], F32, tag="out", name="out_t")

        # preload the Sqrt/Ln ACT table sets during the DMA-fill startup so
        # the final phase doesn't pay the table-load latency.
        warm = pers.tile([P, 2], F32, tag="warm", name="warm")
        nc.vector.memset(warm[:], 1.0)
        SC.activation(warm[:, 0:1], warm[:, 1:2], AF.Sqrt)
        SC.activation(warm[:, 0:1], warm[:, 1:2], AF.Ln)

        x_ap = x_d.ap()
        xt_ap = xt_d.ap()
        g8_ap = g8_d.ap()

        for b in range(NBLK):
            Xb = p_in.tile([P, NCH, T], F16, tag="xb", name="Xb")
            nc.sync.dma_start(out=Xb[:, 0:4], in_=x_ap[b * P:(b + 1) * P, 0:4])
            G8 = p_g8.tile([P, 2, NG, P], F16, tag="g8", name="G8")
            nc.sync.dma_start(out=G8[:], in_=g8_ap[b])
            XT = p_xt.tile([P, 2, 2, P], F16, tag="xt", name="XT")
            nc.sync.dma_start(out=XT[:], in_=xt_ap[b])
            nc.sync.dma_start(out=Xb[:, 4:NCH],
                              in_=x_ap[b * P:(b + 1) * P, 4:NCH])

            # sz bn_stats first: only needs the tiny x[0:4] DMA.
            # interleaved even/odd: even stats = szL (ch 2), odd = szR (ch 3)
            xa = Xb[:, 2, :]
            xi = _bass.AP(tensor=xa.tensor, offset=xa.offset,
                          ap=[xa.ap[0], [1, T], [T, 2]])
            V.add_instruction(mybir.InstBNStats(
                name=f"I-{nc.next_id()}",
                ins=[V.lower_ap(xi)],
                outs=[V.lower_ap(bnS_s[:, b, :])]))

            # gyro squares (only need g8), split by t-chunk so both halves
            # are fully contiguous: chunk 0 on DVE, chunk 1 on ACT
            GSQ = p_gsq.tile([P, 2, NG, P], F16, tag="gsq", name="GSQ")
            V.tensor_tensor(GSQ[:, 0, :, :], G8[:, 0, :, :],
                            G8[:, 0, :, :], op=ALU.mult)
            SC.activation(GSQ[:, 1, :, :], G8[:, 1, :, :], AF.Square)

            # ---------------- PE: DFT + gyro sums ----------------
            psF = []
            for side in range(2):
                ps = p_psf.tile([P, NW], F32, tag=f"psF{side}",
                                name=f"psF{side}")
                for ck in range(2):
                    TE.matmul(ps[:], XT[:, ck, side, :], w_sb[:, ck, :],
                              start=(ck == 0), stop=(ck == 1))
                psF.append(ps)
            psK = p_psk.tile([P, 44], F32, tag="psk", name="psK")
            for c in range(NG):
                for ck in range(2):
                    TE.matmul(psK[:, 8 + c:9 + c], G8[:, ck, c, :], on_sb[:],
                              start=(ck == 0), stop=(ck == 1))
            for c in range(NG):
                for ck in range(2):
                    TE.matmul(psK[:, 26 + c:27 + c], GSQ[:, ck, c, :],
                              on_sb[:], start=(ck == 0), stop=(ck == 1))

            # ---------------- DVE: peaks (abs + max tree) ----------------
            ABS = p_abs.tile([P, NCH, T], F16, tag="absx", name="ABS")
            V.tensor_scalar(ABS[:].bitcast(I16), Xb[:].bitcast(I16),
                            0x7FFF, None, op0=ALU.bitwise_and)
            L1 = p_tree.tile([P, NCH, 128], F16, tag="l1", name="L1")
            V.tensor_tensor(L1[:], ABS[:, :, 0:128], ABS[:, :, 128:256],
                            op=ALU.max)
            L2 = p_tree.tile([P, NCH, 64], F16, tag="l2", name="L2")
            V.tensor_tensor(L2[:], L1[:, :, 0:64], L1[:, :, 64:128],
                            op=ALU.max)
            L3 = p_tree.tile([P, NCH, 32], F16, tag="l3", name="L3")
            V.tensor_tensor(L3[:], L2[:, :, 0:32], L2[:, :, 32:64],
                            op=ALU.max)
            L4 = p_tree.tile([P, NCH, 16], F16, tag="l4", name="L4")
            V.tensor_tensor(L4[:], L3[:, :, 0:16], L3[:, :, 16:32],
                            op=ALU.max)
            # fz/sz peaks -> out cols 0:4 (f_pk L/R, s_pk L/R)
            V.tensor_reduce(out_t[:, b, 0:4], L4[:, 0:4, :], axis=AX.X,
                            op=ALU.max)
            # gyro group peaks -> out cols 22:24, 26:28, 30:32
            vq = out_t[:, b, 20:32].rearrange("p (j q) -> p j q", q=4)
            V.tensor_reduce(vq[:, :, 2:4],
                            L4[:, 4:22, :].rearrange(
                                "p (g c) t -> p g c t", g=6),
                            axis=AX.XY, op=ALU.max)

            # |XT| for half-wave sums, powers of fz (t-layout)
            AXT = p_pow.tile([P, 2, 2, P], F16, tag="axt", name="AXT")
            V.tensor_scalar(AXT[:].bitcast(I16), XT[:].bitcast(I16),
                            0x7FFF, None, op0=ALU.bitwise_and)
            XT2 = p_pow.tile([P, 2, 2, P], F16, tag="xt2", name="XT2")
            SC.activation(XT2[:], XT[:], AF.Square)
            XT3 = p_pow.tile([P, 2, 2, P], F16, tag="xt3", name="XT3")
            V.tensor_tensor(XT3[:], XT2[:], XT[:], op=ALU.mult)
            XT4 = p_pow.tile([P, 2, 2, P], F16, tag="xt4", name="XT4")
            SC.activation(XT4[:], XT2[:], AF.Square)
            for side in range(2):
                for ck in range(2):
                    TE.matmul(psK[:, 2 * ck + side:2 * ck + side + 1],
                              AXT[:, ck, side, :], on_sb[:],
                              start=True, stop=True)
            for side in range(2):
                for ck in range(2):
                    TE.matmul(psK[:, 4 + side:5 + side],
                              XT3[:, ck, side, :], on_sb[:],
                              start=(ck == 0), stop=(ck == 1))
            for side in range(2):
                for ck in range(2):
                    TE.matmul(psK[:, 6 + side:7 + side],
                              XT4[:, ck, side, :], on_sb[:],
                              start=(ck == 0), stop=(ck == 1))

            # ---------------- fz scalar features ----------------
            i2 = 2 * b
            for side in range(2):
                ps = psF[side]
                P2 = p_scr.tile([P, 257], F32, tag="p2", name="P2")
                SC.activation(P2[:], ps[:, 0:257], AF.Square,
                              accum_out=tot_s[:, i2 + side:i2 + side + 1])
                jB = p_junk.tile([P, 255], F32, tag="jb", name="jB")
                SC.activation(jB[:], ps[:, 257:NW], AF.Square,
                              accum_out=scn_s[:, i2 + side:i2 + side + 1])
                # hf: cos^2 k>=60 (cols 60..128) + sin^2 k>=60 (cols 188..255)
                p2b = P2[:]
                hfap = _bass.AP(tensor=p2b.tensor,
                                offset=p2b.offset + HF_BIN,
                                ap=[p2b.ap[0], [P, 2], [1, NBINS - HF_BIN]])
                V.reduce_sum(hf_s[:, i2 + side:i2 + side + 1], hfap,
                             axis=AX.XY)
                # mu and aN from DFT cols 0, 128 (positive scale)
                SC.activation(muN_s[:, 2 * (i2 + side):2 * (i2 + side) + 2],
                              ps[:, 0:NBINS:P], AF.Copy, scale=1.0 / T)

            # impact: sign(|fz| - 0.3 pk), thr per partition
            thr = p_small.tile([P, 2], F32, tag="thr", name="thr")
            V.tensor_scalar(thr[:], out_t[:, b, 0:2], -0.3, None,
                            op0=ALU.mult)
            for side in range(2):
                j1 = p_junk.tile([P, T], F16, tag="j1", name="j1")
                SC.activation(j1[:], ABS[:, side, :], AF.Sign,
                              bias=thr[:, side:side + 1],
                              accum_out=imp_s[:, i2 + side:i2 + side + 1])

            # zcr: sign(x_t * x_{t+1})
            PR = p_scr.tile([P, 2, T - 1], F16, tag="pr", name="PR")
            V.tensor_tensor(PR[:], Xb[:, 0:2, 1:T], Xb[:, 0:2, 0:T - 1],
                            op=ALU.mult)
            for side in range(2):
                j2 = p_junk.tile([P, T], F16, tag="j2", name="j2")
                SC.activation(j2[:, 0:T - 1], PR[:, side, :], AF.Sign,
                              accum_out=zc_s[:, i2 + side:i2 + side + 1])

            # vib: |sz_t - sz_{t-1}|
            D = p_scr.tile([P, 2, T - 1], F16, tag="d", name="D")
            V.tensor_tensor(D[:], Xb[:, 2:4, 1:T], Xb[:, 2:4, 0:T - 1],
                            op=ALU.subtract)
            for side in range(2):
                j3 = p_junk.tile([P, T], F16, tag="j3", name="j3")
                SC.activation(j3[:, 0:T - 1], D[:, side, :], AF.Abs,
                              accum_out=vib_s[:, i2 + side:i2 + side + 1])

            V.tensor_copy(mg_s[:, b, :], psK[:])

        # ================= final batched scalar phase =================
        def v2(tbl):
            return tbl[:].rearrange("p (b s) -> p b s", s=2)

        def ft(tag, shape=(P, NBLK, 2)):
            return fin.tile(list(shape), F32, tag=tag, name=tag)

        # ratio = log1p(f_pk / (s_pk + 1e-4))  [Ln later]
        r_spk = ft("r_spk")
        V.tensor_scalar(r_spk[:], out_t[:, :, 2:4], 1e-4, None, op0=ALU.add)
        V.reciprocal(r_spk[:], r_spk[:])
        ratio_arg = ft("ratio_arg")
        V.tensor_mul(ratio_arg[:], out_t[:, :, 0:2], r_spk[:])

        # total power reciprocal (shared by hf and sc)
        r_tot = ft("r_tot")
        V.tensor_scalar(r_tot[:], v2(tot_s), EPS, None, op0=ALU.add)
        V.reciprocal(r_tot[:], r_tot[:])
        V.tensor_mul(out_t[:, :, 6:8], v2(hf_s), r_tot[:])
        V.scalar_tensor_tensor(out_t[:, :, 16:18], v2(scn_s), 1.0 / NBINS,
                               r_tot[:], op0=ALU.mult, op1=ALU.mult)

        # dur, vib, zcr scaled counts
        V.tensor_scalar(out_t[:, :, 18:20], v2(imp_s), 1.0 / (2 * T), 0.5,
                        op0=ALU.mult, op1=ALU.add)
        V.tensor_scalar(out_t[:, :, 12:14], v2(vib_s), 1.0 / (T - 1), None,
                        op0=ALU.mult)
        V.tensor_scalar(out_t[:, :, 42:44], v2(zc_s), -1.0 / (2 * (T - 1)),
                        0.5, op0=ALU.mult, op1=ALU.add)

        # decay = h0 / (h1 + 128e-6)
        hv = mg_s[:, :, 0:4].rearrange("p b (h s) -> p b h s", h=2)
        dden = ft("dden")
        V.tensor_scalar(dden[:], hv[:, :, 1, :], (T // 2) * EPS, None,
                        op0=ALU.add)
        V.reciprocal(dden[:], dden[:])
        V.tensor_mul(out_t[:, :, 10:12], hv[:, :, 0, :], dden[:])

        # asym_acc = |f_pk - s_pk|
        aa = ft("aa")
        V.tensor_sub(aa[:], out_t[:, :, 0:2], out_t[:, :, 2:4])
        SC.activation(out_t[:, :, 32:34], aa[:], AF.Abs)

        # foot Sx2 via Parseval: sqF = tot/128 - 256 (mu^2 + aN^2)
        mv = muN_s[:].rearrange("p (b s h) -> p b s h", s=2, h=2)
        qF = ft("qF")
        V.tensor_mul(qF[:], mv[:, :, :, 0], mv[:, :, :, 0])
        qN = ft("qN")
        V.tensor_mul(qN[:], mv[:, :, :, 1], mv[:, :, :, 1])
        qsum = ft("qsum")
        V.tensor_add(qsum[:], qF[:], qN[:])
        tq = ft("tq")
        V.tensor_scalar(tq[:], v2(tot_s), 1.0 / (T // 2), None, op0=ALU.mult)
        sqF = ft("sqF")           # sum x^2
        V.scalar_tensor_tensor(sqF[:], qsum[:], -float(T), tq[:],
                               op0=ALU.mult, op1=ALU.add)
        m2F = ft("m2F")           # sum (x-mu)^2
        V.scalar_tensor_tensor(m2F[:], qF[:], -float(T), sqF[:],
                               op0=ALU.mult, op1=ALU.add)

        # central moments from raw sums:
        # m3c = Sx3 - 3 mu sqF + 2 T mu^3
        # m4c = Sx4 - 4 mu Sx3 + 6 mu^2 sqF - 3 T mu^4
        mu = mv[:, :, :, 0]
        Sx3 = mg_s[:, :, 4:6]
        Sx4 = mg_s[:, :, 6:8]
        mu2 = qF
        muSq = ft("muSq")                       # mu * sqF
        V.tensor_mul(muSq[:], mu, sqF[:])
        mu3 = ft("mu3")
        V.tensor_mul(mu3[:], mu2[:], mu)
        t1 = ft("t1")                           # Sx3 - 3 mu sqF
        V.scalar_tensor_tensor(t1[:], muSq[:], -3.0, Sx3,
                               op0=ALU.mult, op1=ALU.add)
        m3c = ft("m3c")
        V.scalar_tensor_tensor(m3c[:], mu3[:], 2.0 * T, t1[:],
                               op0=ALU.mult, op1=ALU.add)
        muSx3 = ft("muSx3")
        V.tensor_mul(muSx3[:], mu, Sx3)
        mu2sq = ft("mu2sq")                     # mu^2 * sqF
        V.tensor_mul(mu2sq[:], mu2[:], sqF[:])
        mu4 = ft("mu4")
        V.tensor_mul(mu4[:], mu2[:], mu2[:])
        t2 = ft("t2")                           # Sx4 - 4 mu Sx3
        V.scalar_tensor_tensor(t2[:], muSx3[:], -4.0, Sx4,
                               op0=ALU.mult, op1=ALU.add)
        t3 = ft("t3")                           # + 6 mu^2 sqF
        V.scalar_tensor_tensor(t3[:], mu2sq[:], 6.0, t2[:],
                               op0=ALU.mult, op1=ALU.add)
        m4c = ft("m4c")
        V.scalar_tensor_tensor(m4c[:], mu4[:], -3.0 * T, t3[:],
                               op0=ALU.mult, op1=ALU.add)

        # shank stats from bn call 0: even = szL, odd = szR
        meS = bnS_s[:, :, 1:5:3]               # [p, b, 2] means (cols 1,4)
        M2S = bnS_s[:, :, 2:6:3]               # [p, b, 2] sum (y-mu)^2 (cols 2,5)
        sqS = ft("sqS")                         # sum y^2 = M2 + T mu^2
        uS = ft("uS")
        V.tensor_mul(uS[:], meS, meS)
        V.scalar_tensor_tensor(sqS[:], uS[:], float(T), M2S,
                               op0=ALU.mult, op1=ALU.add)

        # var_ratio = log1p(m2F / (M2S + 255e-4))  [Ln later]
        vr = ft("vr")
        V.tensor_scalar(vr[:], M2S, (T - 1) * 1e-4, None, op0=ALU.add)
        V.reciprocal(vr[:], vr[:])
        vra = ft("vra")
        V.tensor_mul(vra[:], m2F[:], vr[:])

        # gyro M2 group sums: calls 1..9, even col (2) = lt, odd col (5) = rt
        # gyro M2 group sums from the PE Sx/Sxx stats:
        # M2_c = Sxx_c - Sx_c^2 / T; groups are 3 consecutive channels.
        gSx = mg_s[:, :, 8:26]
        gSxx = mg_s[:, :, 26:44]
        gsq = ft("gsq_g", (P, NBLK, NG))
        V.tensor_mul(gsq[:], gSx, gSx)
        gm2c = ft("gm2c", (P, NBLK, NG))
        V.scalar_tensor_tensor(gm2c[:], gsq[:], -1.0 / T, gSxx,
                               op0=ALU.mult, op1=ALU.add)
        gM2 = ft("gM2", (P, NBLK, 3, 2))
        V.reduce_sum(gM2[:].rearrange("p b g s -> p b (g s)"),
                     gm2c[:].rearrange("p b (j c) -> p b j c", c=3),
                     axis=AX.X)

        vq = out_t[:, :, 20:32].rearrange("p b (j q) -> p b j q", q=4)

        # ---- Sqrt-set ACT ops ----
        SC.activation(out_t[:, :, 8:10], m2F[:], AF.Sqrt, scale=1.0 / (T - 1))
        rmsF = ft("rmsF")
        SC.activation(rmsF[:], sqF[:], AF.Sqrt, scale=1.0 / T)
        rmsS = ft("rmsS")
        SC.activation(rmsS[:], sqS[:], AF.Sqrt, scale=1.0 / T)

        # kurt/skew (uses std at out[...,8:10])
        sg = ft("sg")
        V.tensor_scalar(sg[:], out_t[:, :, 8:10], 1e-6, None, op0=ALU.max)
        vv = ft("vv")
        V.tensor_mul(vv[:], sg[:], sg[:])
        v4 = ft("v4")
        V.tensor_mul(v4[:], vv[:], vv[:])
        V.reciprocal(v4[:], v4[:])
        kr = ft("kr")
        V.scalar_tensor_tensor(kr[:], m4c[:], 1.0 / T, v4[:],
                               op0=ALU.mult, op1=ALU.mult)
        V.tensor_scalar(out_t[:, :, 38:40], kr[:], 30.0, -10.0,
                        op0=ALU.min, op1=ALU.max)
        v3 = ft("v3")
        V.tensor_mul(v3[:], vv[:], sg[:])
        V.reciprocal(v3[:], v3[:])
        sk = ft("sk")
        V.scalar_tensor_tensor(sk[:], m3c[:], 1.0 / T, v3[:],
                               op0=ALU.mult, op1=ALU.mult)
        V.tensor_scalar(out_t[:, :, 40:42], sk[:], 10.0, -10.0,
                        op0=ALU.min, op1=ALU.max)

        # trans arg = rmsS / (rmsF + 1e-6)
        rdn = ft("rdn")
        V.tensor_scalar(rdn[:], rmsF[:], EPS, None, op0=ALU.add)
        V.reciprocal(rdn[:], rdn[:])
        targ = ft("targ")
        V.tensor_mul(targ[:], rmsS[:], rdn[:])

        # ---- Ln-set ACT ops (log1p via bias=1) ----
        SC.activation(out_t[:, :, 4:6], ratio_arg[:], AF.Ln, bias=1.0)
        SC.activation(out_t[:, :, 14:16], vra[:], AF.Ln, bias=1.0)
        SC.activation(out_t[:, :, 36:38], targ[:], AF.Ln, bias=1.0)
        SC.activation(vq[:, :, :, 0:2], gM2[:],
                      AF.Ln, scale=1.0 / (T - 1), bias=1.0)

        # asym_gy = |fg_var - sg_var| (after log1p)
        ag = ft("ag")
        V.tensor_sub(ag[:], out_t[:, :, 20:22], out_t[:, :, 24:26])
        SC.activation(out_t[:, :, 34:36], ag[:], AF.Abs)

        # ---- store ----
        nc.sync.dma_start(out=out_d.ap().rearrange("(b p) f -> p b f", p=P),
                          in_=out_t[:])


_NC_CACHE = None
_CONSTS = None


def _get_nc():
    global _NC_CACHE, _CONSTS
    if _NC_CACHE is None:
        _NC_CACHE = build_nc()
    if _CONSTS is None:
        _CONSTS = build_consts()
    return _NC_CACHE, _CONSTS


def run(foot, shank, thigh, **kw):
    arrs = {"foot": np.asarray(foot), "shank": np.asarray(shank),
            "thigh": np.asarray(thigh)}
    X = np.empty((B_FULL, NCH, T), dtype=np.float16)
    for j, (name, ch) in enumerate(SRC):
        X[:, j] = arrs[name][:, ch]

    # fz t-layout: xt[blk, t, chunk, side, s] per core
    import ml_dtypes
    nc, consts = _get_nc()
    in_maps = []
    for i in range(N_CORES):
        Xc = X[i * BC:(i + 1) * BC]
        fz = Xc[:, 0:2, :].reshape(NBLK, P, 2, 2, P)   # [b, s, side, ck, t]
        xt = np.ascontiguousarray(fz.transpose(0, 4, 3, 2, 1))
        gy = Xc[:, 4:22, :].reshape(NBLK, P, NG, 2, P)  # [b, s, c, ck, t]
        g8 = np.ascontiguousarray(gy.transpose(0, 4, 3, 2, 1))
        in_maps.append({
            "x": np.ascontiguousarray(Xc),
            "xt": xt,
            "g8": g8,
            "w": consts["w"], "ones": consts["ones"],
            "ones8": consts["ones8"],
        })
    return run_bass_kernel_spmd(nc, in_maps, core_ids=list(range(N_CORES)),
                                **kw)


def kernel(foot, shank, thigh):
    res = run(foot, shank, thigh)
    return np.concatenate([res.results[i]["out"] for i in range(N_CORES)],
                          axis=0)
